# revision 3
# baseline (speedup 1.0000x reference)
"""MLA (multi-head latent attention) Trainium2 Bass kernel, v2.

Sharding: 8 cores = batch(2) x head-groups(4 heads each).
v2 replicates the latent projections (x @ W_dq / W_dkv) on every core of a
batch group instead of all-gathering latent quarters: the AllGather's
fixed+bandwidth cost exceeded the extra 110us of PE time, and removing it
also removes the cross-core sync point.

The whole kernel is one interleaved pipeline over 4 sequence chunks:
  chunk sn: latents(sn) + k_rope(sn) -> q/k/v up-proj(sn) -> attention(qb=sn)
so attention's Act/DVE work overlaps the projection matmuls and PE never
waits on a phase barrier.

All matmuls run in bf16 (full PE rate) with fp32 PSUM accumulation.
Attention is computed in "transposed score" orientation S_T[j, q] so the
softmax denominator folds into the PE via a ones-column appended to V and
no cross-partition reductions are needed. Softmax skips max-subtraction
(scores are O(1) here; exp is computed in fp32 from PSUM).
"""

import sys
import numpy as np
import ml_dtypes

for _p in ("/opt/trn_rl_repo", "/root/.axon_site/_ro/trn_rl_repo"):
    if _p not in sys.path:
        sys.path.append(_p)

BF16 = ml_dtypes.bfloat16

D_MODEL = 2048
SEQ = 2048
BATCH = 2
N_HEADS = 16
D_HEAD = 128
D_KV = 512
D_ROPE = 64
ROPE_BASE = 10000.0
EPS = 1e-5
H_LOC = 4          # heads per core
N_CORES = 8

_BUILD_CACHE = {}


def build_program(reps: int = 1):
    """Build (and cache) the per-core Bass program. SPMD: same program on
    all 8 cores; per-core data differs via the input maps."""
    if reps in _BUILD_CACHE:
        return _BUILD_CACHE[reps]

    import concourse.bass as bass  # noqa: F401
    import concourse.mybir as mybir
    from concourse import bacc
    from concourse.tile import TileContext
    from concourse.masks import make_identity
    from contextlib import ExitStack

    f32 = mybir.dt.float32
    bf16 = mybir.dt.bfloat16
    AF = mybir.ActivationFunctionType
    OP = mybir.AluOpType

    nc = bacc.Bacc(num_devices=8)

    xT = nc.declare_dram_parameter("xT", [D_MODEL, SEQ], bf16, isOutput=False)
    wdq = nc.declare_dram_parameter("wdq", [D_MODEL, D_KV], bf16, isOutput=False)
    wdkv = nc.declare_dram_parameter("wdkv", [D_MODEL, D_KV], bf16, isOutput=False)
    wq = nc.declare_dram_parameter("wq", [D_KV, H_LOC * 128], bf16, isOutput=False)
    wuk2 = nc.declare_dram_parameter("wuk2", [D_KV, 2 * 128], bf16, isOutput=False)
    wkr2 = nc.declare_dram_parameter("wkr2", [D_MODEL, 2 * 128], bf16, isOutput=False)
    wuv = nc.declare_dram_parameter("wuv", [D_KV, H_LOC * 128], bf16, isOutput=False)
    wout = nc.declare_dram_parameter("wout", [H_LOC * 128, D_MODEL], bf16, isOutput=False)
    mult = nc.declare_dram_parameter("mult", [128, 2 * SEQ], bf16, isOutput=False)
    masks = nc.declare_dram_parameter("masks", [128, 4 * 512], bf16, isOutput=False)
    y = nc.declare_dram_parameter("y", [SEQ, D_MODEL], bf16, isOutput=True)

    SCALE = 1.0 / float(np.sqrt(np.float32(D_HEAD)))
    NKT = D_MODEL // 128    # 16 k-tiles over d_model
    NLT = D_KV // 128       # 4  k-tiles over latent
    NSN = SEQ // 512        # 4  sequence chunks
    VROW = D_HEAD + 1       # 129: V row with ones column

    with TileContext(nc) as tc, ExitStack() as top:
        pp = top.enter_context(tc.tile_pool(name="persist", bufs=1))
        # persistent SBUF tensors (live until the end)
        kt_sb = pp.tile([128, H_LOC * SEQ], bf16, tag="kt")
        v_sb = pp.tile([128, (SEQ // 128) * H_LOC * VROW], bf16, tag="v")
        wdq_sb = pp.tile([128, NKT * D_KV], bf16, tag="wdq")
        wdkv_sb = pp.tile([128, NKT * D_KV], bf16, tag="wdkv")
        wkr2_sb = pp.tile([128, NKT * 256], bf16, tag="wkr2")
        wq_sb = pp.tile([128, NLT * 512], bf16, tag="wq")
        wuk2_sb = pp.tile([128, NLT * 256], bf16, tag="wuk2")
        wuv_sb = pp.tile([128, NLT * 512], bf16, tag="wuv")
        wout_sb = pp.tile([128, NLT * D_MODEL], bf16, tag="wout")
        mult_sb = pp.tile([128, 2 * SEQ], bf16, tag="mult")
        masks_sb = pp.tile([128, 4 * 512], bf16, tag="masks")
        ident_sb = pp.tile([128, 128], bf16, tag="ident")
        ones_sb = pp.tile([128, 1], bf16, tag="ones")
        eps_sb = pp.tile([1, 1], f32, tag="eps")

        for _rep in range(reps):
            with ExitStack() as body:
                pX = body.enter_context(tc.tile_pool(name="pX", bufs=2))
                pCq = body.enter_context(tc.tile_pool(name="pCq", bufs=2))
                pCkv = body.enter_context(tc.tile_pool(name="pCkv", bufs=2))
                pCp = body.enter_context(tc.tile_pool(name="pCp", bufs=4))
                pSq = body.enter_context(tc.tile_pool(name="pSq", bufs=3))
                pSt = body.enter_context(tc.tile_pool(name="pSt", bufs=3))
                pQt = body.enter_context(tc.tile_pool(name="pQt", bufs=2))
                pEs = body.enter_context(tc.tile_pool(name="pEs", bufs=6))
                pOd = body.enter_context(tc.tile_pool(name="pOd", bufs=4))
                pOt = body.enter_context(tc.tile_pool(name="pOt", bufs=8))
                pYs = body.enter_context(tc.tile_pool(name="pYs", bufs=3))
                psMM = body.enter_context(tc.tile_pool(name="psMM", bufs=3, space="PSUM"))
                psS = body.enter_context(tc.tile_pool(name="psS", bufs=1, space="PSUM"))
                psO = body.enter_context(tc.tile_pool(name="psO", bufs=1, space="PSUM"))

                # ---- weight / constant loads, priority order: the first
                # latent chains need wdq+wdkv; keep those and x on the two
                # fast HWDGE queues (sync/scalar), everything else on gpsimd.
                xch0 = pX.tile([128, NKT * 512], bf16, tag="xs", name="xs0")
                nc.gpsimd.dma_start(out=mult_sb[:], in_=mult[:, :])
                for lt in range(NLT):
                    nc.gpsimd.dma_start(out=wq_sb[:, lt * 512:(lt + 1) * 512],
                                        in_=wq[lt * 128:(lt + 1) * 128, :])
                    nc.gpsimd.dma_start(out=wuk2_sb[:, lt * 256:(lt + 1) * 256],
                                        in_=wuk2[lt * 128:(lt + 1) * 128, :])
                    nc.gpsimd.dma_start(out=wuv_sb[:, lt * 512:(lt + 1) * 512],
                                        in_=wuv[lt * 128:(lt + 1) * 128, :])
                nc.gpsimd.dma_start(out=masks_sb[:], in_=masks[:, :])
                for kt in range(NKT):
                    nc.sync.dma_start(out=wdq_sb[:, kt * D_KV:(kt + 1) * D_KV],
                                      in_=wdq[kt * 128:(kt + 1) * 128, :])
                    eng = (nc.sync, nc.scalar)[kt % 2]
                    eng.dma_start(out=xch0[:, kt * 512:(kt + 1) * 512],
                                  in_=xT[kt * 128:(kt + 1) * 128, 0:512])
                    nc.scalar.dma_start(out=wdkv_sb[:, kt * D_KV:(kt + 1) * D_KV],
                                        in_=wdkv[kt * 128:(kt + 1) * 128, :])
                    nc.gpsimd.dma_start(out=wkr2_sb[:, kt * 256:(kt + 1) * 256],
                                        in_=wkr2[kt * 128:(kt + 1) * 128, :])
                for f in range(NLT):
                    nc.scalar.dma_start(out=wout_sb[:, f * D_MODEL:(f + 1) * D_MODEL],
                                        in_=wout[f * 128:(f + 1) * 128, :])
                nc.gpsimd.memset(ones_sb[:], 1.0)
                nc.gpsimd.memset(eps_sb[:], EPS)
                make_identity(nc, ident_sb[:])
                # ones columns of V (d column 128 of each 129-wide row block)
                v_ones = v_sb.rearrange("p (k d) -> p k d", d=VROW)[:, :, 128:129]
                nc.vector.memset(v_ones, 1.0)

                for sn in range(NSN):
                    s0, s1 = sn * 512, (sn + 1) * 512

                    # ---- x slices for this chunk (streamed, 16 x [128,512];
                    # sn=0's were prefetched with the weight stream) ----
                    if sn == 0:
                        xch = xch0
                    else:
                        xch = pX.tile([128, NKT * 512], bf16, tag="xs", name=f"xs{sn}")
                        for kt in range(NKT):
                            eng = (nc.sync, nc.scalar)[kt % 2]
                            eng.dma_start(out=xch[:, kt * 512:(kt + 1) * 512],
                                          in_=xT[kt * 128:(kt + 1) * 128, s0:s1])
                    xs = [xch[:, kt * 512:(kt + 1) * 512] for kt in range(NKT)]

                    # ---- latents c_q / c_kv for this chunk, rmsnorm ----
                    cq_cur = pCq.tile([128, NLT * 512], bf16, tag="cq", name=f"cq{sn}")
                    ckv_cur = pCkv.tile([128, NLT * 512], bf16, tag="ckv", name=f"ckv{sn}")
                    for ci, (cname, wd_sb, cfull) in enumerate(
                            (("q", wdq_sb, cq_cur), ("kv", wdkv_sb, ckv_cur))):
                        cps_l, sq_l = [], []
                        for lt in range(NLT):
                            cp = psMM.tile([128, 512], f32, tag="mm")
                            for kt in range(NKT):
                                nc.tensor.matmul(
                                    cp[:],
                                    wd_sb[:, kt * D_KV + lt * 128: kt * D_KV + (lt + 1) * 128],
                                    xs[kt],
                                    start=(kt == 0), stop=(kt == NKT - 1))
                            cps = pCp.tile([128, 512], f32, tag="cpre", name=f"cpre{cname}{sn}_{lt}")
                            nc.vector.tensor_copy(cps[:], cp[:])
                            sq = pSq.tile([128, 512], bf16, tag="sq", name=f"sq{cname}{sn}_{lt}")
                            nc.vector.tensor_tensor(sq[:], cps[:], cps[:], OP.mult)
                            cps_l.append(cps)
                            sq_l.append(sq)
                        ss = psS.tile([1, 512], f32, tag="stat")
                        for lt in range(NLT):
                            nc.tensor.matmul(ss[:], ones_sb[:], sq_l[lt][:],
                                             start=(lt == 0), stop=(lt == NLT - 1))
                        # rstd = (ss/512 + eps)^-0.5 via Newton on DVE
                        # (keeps the Act engine exp-only: no act-table swaps).
                        # v is concentrated near 1, so y0=1 and 3 iterations
                        # reach ~1e-4 relative error.
                        v_t = pSt.tile([1, 512], f32, tag="st1")
                        nc.vector.tensor_scalar(v_t[:], ss[:], 1.0 / D_KV, EPS,
                                                OP.mult, OP.add)
                        yy = pSt.tile([1, 512], f32, tag="st1")
                        nc.vector.tensor_scalar(yy[:], ss[:], -0.5 / D_KV,
                                                1.5 - 0.5 * EPS, OP.mult, OP.add)
                        tsq = pSt.tile([1, 512], f32, tag="st1")
                        for _it in range(2):
                            nc.vector.tensor_tensor(tsq[:], yy[:], yy[:], OP.mult)
                            nc.vector.tensor_tensor(tsq[:], tsq[:], v_t[:], OP.mult)
                            nc.vector.tensor_scalar(tsq[:], tsq[:], -0.5, 1.5,
                                                    OP.mult, OP.add)
                            nc.vector.tensor_tensor(yy[:], yy[:], tsq[:], OP.mult)
                        bstd = pSt.tile([128, 512], f32, tag="bstd")
                        nc.gpsimd.partition_broadcast(bstd[:], yy[:])
                        for lt in range(NLT):
                            nc.vector.tensor_tensor(
                                cfull[:, lt * 512:(lt + 1) * 512],
                                cps_l[lt][:], bstd[:], OP.mult)

                    # ---- k_rope for this chunk (from x, not latents) ----
                    for p in range(2):
                        kp = psMM.tile([128, 512], f32, tag="mm", name=f"kr{sn}_{p}")
                        for kt in range(NKT):
                            nc.tensor.matmul(
                                kp[:],
                                wkr2_sb[:, kt * 256 + p * 128: kt * 256 + (p + 1) * 128],
                                xs[kt],
                                start=(kt == 0), stop=(kt == NKT - 1))
                        h0, h1 = 2 * p, 2 * p + 1
                        m0 = mult_sb[:, 0 * SEQ + s0: 0 * SEQ + s1]
                        m1 = mult_sb[:, 1 * SEQ + s0: 1 * SEQ + s1]
                        k0 = kt_sb[:, h0 * SEQ + s0: h0 * SEQ + s1]
                        k1 = kt_sb[:, h1 * SEQ + s0: h1 * SEQ + s1]
                        nc.vector.tensor_tensor(k0[64:128, :], kp[64:128, :], m0[64:128, :], OP.mult)
                        nc.vector.tensor_tensor(k1[0:64, :], kp[0:64, :], m1[0:64, :], OP.mult)

                    # ---- q/k/v up-projections for this chunk ----
                    def cnq(lt):
                        return cq_cur[:, lt * 512:(lt + 1) * 512]

                    def cnkv(lt):
                        return ckv_cur[:, lt * 512:(lt + 1) * 512]

                    qt_cur = pQt.tile([128, H_LOC * 512], bf16, tag="qt", name=f"qt{sn}")
                    for hl in range(H_LOC):
                        qp = psMM.tile([128, 512], f32, tag="mm")
                        for lt in range(NLT):
                            nc.tensor.matmul(
                                qp[:],
                                wq_sb[:, lt * 512 + hl * 128: lt * 512 + (hl + 1) * 128],
                                cnq(lt),
                                start=(lt == 0), stop=(lt == NLT - 1))
                        nc.vector.tensor_tensor(
                            qt_cur[:, hl * 512:(hl + 1) * 512],
                            qp[:], mult_sb[:, (hl % 2) * SEQ + s0: (hl % 2) * SEQ + s1],
                            OP.mult)
                    for p in range(2):
                        h0, h1 = 2 * p, 2 * p + 1
                        up = psMM.tile([128, 512], f32, tag="mm")
                        for lt in range(NLT):
                            nc.tensor.matmul(
                                up[:],
                                wuk2_sb[:, lt * 256 + p * 128: lt * 256 + (p + 1) * 128],
                                cnkv(lt),
                                start=(lt == 0), stop=(lt == NLT - 1))
                        m0 = mult_sb[:, 0 * SEQ + s0: 0 * SEQ + s1]
                        m1 = mult_sb[:, 1 * SEQ + s0: 1 * SEQ + s1]
                        k0 = kt_sb[:, h0 * SEQ + s0: h0 * SEQ + s1]
                        k1 = kt_sb[:, h1 * SEQ + s0: h1 * SEQ + s1]
                        nc.vector.tensor_tensor(k0[0:64, :], up[0:64, :], m0[0:64, :], OP.mult)
                        nc.vector.tensor_tensor(k1[64:128, :], up[64:128, :], m1[64:128, :], OP.mult)
                    for st in range(4):
                        s_tile = sn * 4 + st
                        vp = psMM.tile([128, 512], f32, tag="mm")
                        for lt in range(NLT):
                            nc.tensor.matmul(
                                vp[:],
                                cnkv(lt)[:, st * 128:(st + 1) * 128],
                                wuv_sb[:, lt * 512:(lt + 1) * 512],
                                start=(lt == 0), stop=(lt == NLT - 1))
                        vdst = v_sb.rearrange("p (k d) -> p k d", d=VROW)[
                            :, s_tile * H_LOC:(s_tile + 1) * H_LOC, 0:128]
                        vsrc = vp.rearrange("p (k d) -> p k d", d=128)
                        nc.vector.tensor_copy(vdst, vsrc)

                    # ---- attention for q-block qb == sn ----
                    qb = sn
                    njt = (qb + 1) * 4
                    otc = [pOt.tile([128, 512], bf16, tag="otc", name=f"otc{qb}_{f}")
                           for f in range(H_LOC)]
                    for hl in range(H_LOC):
                        # ob: 4 accumulation regions (qs), one PSUM bank each
                        ob = psO.tile([128, 2048], f32, tag="obank")
                        # full j-tiles (strictly below the diagonal band)
                        for jt in range(qb * 4):
                            sp = psMM.tile([128, 512], f32, tag="mm", name=f"s{qb}{hl}{jt}")
                            nc.tensor.matmul(
                                sp[:],
                                kt_sb[:, hl * SEQ + jt * 128: hl * SEQ + (jt + 1) * 128],
                                qt_cur[:, hl * 512:(hl + 1) * 512],
                                start=True, stop=True)
                            es = pEs.tile([128, 512], bf16, tag="expS", name=f"e{qb}{hl}{jt}")
                            nc.scalar.activation(es[:], sp[:], AF.Exp, scale=SCALE)
                            vsl = v_sb[:, jt * H_LOC * VROW + hl * VROW:
                                       jt * H_LOC * VROW + (hl + 1) * VROW]
                            for qs in range(4):
                                nc.tensor.matmul(
                                    ob[:, qs * 512: qs * 512 + VROW],
                                    es[:, qs * 128:(qs + 1) * 128],
                                    vsl,
                                    start=(jt == 0), stop=False)
                        # diagonal j-tiles (kd = 0..3)
                        for kd in range(4):
                            jt = qb * 4 + kd
                            c0 = kd * 128
                            sp = psMM.tile([128, 512], f32, tag="mm", name=f"s{qb}{hl}{jt}")
                            nc.tensor.matmul(
                                sp[:, c0:],
                                kt_sb[:, hl * SEQ + jt * 128: hl * SEQ + (jt + 1) * 128],
                                qt_cur[:, hl * 512 + c0:(hl + 1) * 512],
                                start=True, stop=True)
                            es = pEs.tile([128, 512], bf16, tag="expS", name=f"e{qb}{hl}{jt}")
                            nc.scalar.activation(es[:, c0:], sp[:, c0:], AF.Exp, scale=SCALE)
                            nc.vector.tensor_tensor(
                                es[:, c0:], es[:, c0:],
                                masks_sb[:, kd * 512 + c0:(kd + 1) * 512], OP.mult)
                            vsl = v_sb[:, jt * H_LOC * VROW + hl * VROW:
                                       jt * H_LOC * VROW + (hl + 1) * VROW]
                            for qs in range(kd, 4):
                                nc.tensor.matmul(
                                    ob[:, qs * 512: qs * 512 + VROW],
                                    es[:, qs * 128:(qs + 1) * 128],
                                    vsl,
                                    start=(jt == 0), stop=(jt == qb * 4 + qs))
                        for qs in range(4):
                            zr = pOd.tile([128, 1], f32, tag="zr")
                            nc.vector.reciprocal(zr[:], ob[:, qs * 512 + 128: qs * 512 + VROW])
                            od = pOd.tile([128, 128], bf16, tag="od")
                            nc.vector.tensor_scalar_mul(od[:], ob[:, qs * 512: qs * 512 + 128], zr[:])
                            tp = psMM.tile([128, 128], bf16, tag="mm", name=f"tp{qb}{hl}{qs}")
                            nc.tensor.transpose(tp[:], od[:], ident_sb[:])
                            nc.vector.tensor_copy(otc[hl][:, qs * 128:(qs + 1) * 128], tp[:])

                    # ---- output projection for this q-block ----
                    for st in range(4):
                        row0 = qb * 512 + st * 128
                        for ncol in range(4):
                            yp = psMM.tile([128, 512], f32, tag="mm", name=f"y{qb}{st}{ncol}")
                            for f in range(H_LOC):
                                nc.tensor.matmul(
                                    yp[:],
                                    otc[f][:, st * 128:(st + 1) * 128],
                                    wout_sb[:, f * D_MODEL + ncol * 512: f * D_MODEL + (ncol + 1) * 512],
                                    start=(f == 0), stop=(f == H_LOC - 1))
                            ys = pYs.tile([128, 512], bf16, tag="ysb")
                            if (st + ncol) % 2 == 0:
                                nc.vector.tensor_copy(ys[:], yp[:])
                            else:
                                nc.scalar.copy(ys[:], yp[:])
                            nc.sync.dma_start(out=y[row0:row0 + 128, ncol * 512:(ncol + 1) * 512],
                                              in_=ys[:])

    nc.finalize()
    _BUILD_CACHE[reps] = nc
    return nc


def _rope_mult():
    """r[s, d] = cos + sin rope multiplier, transposed to [64, SEQ]."""
    half = D_ROPE // 2
    theta = 1.0 / (ROPE_BASE ** (np.arange(0, D_HEAD, 2, dtype=np.float32) / D_HEAD))
    idx = np.arange(SEQ, dtype=np.float32)[:, None] * theta[None, :]
    r = np.tile(np.cos(idx[:, :half]), (1, 2)) + np.tile(np.sin(idx[:, :half]), (1, 2))
    return np.ascontiguousarray(r.T).astype(np.float32)  # [64, SEQ]


def make_inputs(x, W_dq, W_uq, W_dkv, W_uk, W_uv, W_qr, W_kr, g_q, g_kv, W_out, b_out):
    """Host-side sharding/packing: per-core input maps."""
    rT = _rope_mult()
    mult = np.empty((128, 2 * SEQ), np.float32)
    mult[0:64, 0:SEQ] = 1.0
    mult[64:128, 0:SEQ] = rT
    mult[0:64, SEQ:] = rT
    mult[64:128, SEQ:] = 1.0
    mult = mult.astype(BF16)

    masks = np.zeros((128, 4 * 512), np.float32)
    jl = np.arange(128)[:, None]
    ql = np.arange(512)[None, :]
    for k in range(4):
        masks[:, k * 512:(k + 1) * 512] = (ql >= 128 * k + jl)
    masks = masks.astype(BF16)

    gq = g_q.astype(np.float32)[:, None]
    gkv = g_kv.astype(np.float32)[:, None]
    Wuq_g = W_uq * gq
    Wqr_g = W_qr * gq
    Wuk_g = W_uk * gkv
    Wuv_g = W_uv * gkv

    in_maps = []
    for core in range(N_CORES):
        b = core // 4
        g = core % 4
        heads = [4 * g + i for i in range(H_LOC)]

        xb = np.ascontiguousarray(x[b].T).astype(BF16)  # [d_model, seq]

        wq_pack = np.empty((D_KV, H_LOC * 128), np.float32)
        for hl, h in enumerate(heads):
            a = Wuq_g[:, h * 64:(h + 1) * 64]
            r = Wqr_g[:, h * 64:(h + 1) * 64]
            blk = np.concatenate([a, r], axis=1) if hl % 2 == 0 else np.concatenate([r, a], axis=1)
            wq_pack[:, hl * 128:(hl + 1) * 128] = blk

        wuk2 = np.empty((D_KV, 256), np.float32)
        wkr2 = np.empty((D_MODEL, 256), np.float32)
        for p in range(2):
            h0, h1 = heads[2 * p], heads[2 * p + 1]
            wuk2[:, p * 128: p * 128 + 64] = Wuk_g[:, h0 * 64:(h0 + 1) * 64]
            wuk2[:, p * 128 + 64: p * 128 + 128] = Wuk_g[:, h1 * 64:(h1 + 1) * 64]
            # rot halves swapped: odd head's rope block first
            wkr2[:, p * 128: p * 128 + 64] = W_kr[:, h1 * 64:(h1 + 1) * 64]
            wkr2[:, p * 128 + 64: p * 128 + 128] = W_kr[:, h0 * 64:(h0 + 1) * 64]

        wuv_pack = np.concatenate(
            [Wuv_g[:, h * 128:(h + 1) * 128] for h in heads], axis=1)
        wout_pack = np.concatenate(
            [W_out[h * 128:(h + 1) * 128, :] for h in heads], axis=0)

        in_maps.append({
            "xT": xb,
            "wdq": W_dq.astype(BF16),
            "wdkv": W_dkv.astype(BF16),
            "wq": wq_pack.astype(BF16),
            "wuk2": wuk2.astype(BF16),
            "wkr2": wkr2.astype(BF16),
            "wuv": wuv_pack.astype(BF16),
            "wout": wout_pack.astype(BF16),
            "mult": mult,
            "masks": masks,
        })
    return in_maps


def kernel(**inputs):
    inputs = {k: np.asarray(v) for k, v in inputs.items()}
    in_maps = make_inputs(
        inputs["x"], inputs["W_dq"], inputs["W_uq"], inputs["W_dkv"],
        inputs["W_uk"], inputs["W_uv"], inputs["W_qr"], inputs["W_kr"],
        inputs["g_q"], inputs["g_kv"], inputs["W_out"], inputs["b_out"])

    nc = build_program(reps=1)
    from concourse.bass_utils import run_bass_kernel_spmd
    res = run_bass_kernel_spmd(nc, in_maps, list(range(N_CORES)))

    b_out = inputs["b_out"].astype(np.float32)
    out = np.zeros((BATCH, SEQ, D_MODEL), np.float32)
    for core in range(N_CORES):
        out[core // 4] += res.results[core]["y"].astype(np.float32)
    out += b_out[None, None, :]
    return out


# revision 4
# speedup vs baseline: 1.8014x; 1.8014x over previous
"""MLA (multi-head latent attention) Trainium2 Bass kernel, v3.

Sharding: 8 cores = batch(2) x head-groups(4 heads each). Latent
projections replicated per core (no collective -> no mesh sync, immune to
pipelined-execution desync). One interleaved pipeline over 4 sequence
chunks: latents(sn)+k_rope(sn) -> q/k/v up-proj(sn) -> attention(qb=sn).

v3 packs almost all PSUM work into [128,1024] two-bank pair tiles (one
accumulation group per bank), halving instruction counts on PE/Act/DVE:
exp runs once per score PAIR, latent drains copy 1024 columns at a time,
and attention AV runs qs-major in single-bank chains so the softmax
denominator still folds into the PE via a ones-column on V.

rmsnorm's rsqrt is computed with a 3-step Newton iteration on the DVE
(v ~ 1), keeping the Act engine exp-only: one activation table load total.
"""

import sys
import numpy as np
import ml_dtypes

for _p in ("/opt/trn_rl_repo", "/root/.axon_site/_ro/trn_rl_repo"):
    if _p not in sys.path:
        sys.path.append(_p)

BF16 = ml_dtypes.bfloat16

D_MODEL = 2048
SEQ = 2048
BATCH = 2
N_HEADS = 16
D_HEAD = 128
D_KV = 512
D_ROPE = 64
ROPE_BASE = 10000.0
EPS = 1e-5
H_LOC = 4          # heads per core
N_CORES = 8

_BUILD_CACHE = {}


def build_program(reps: int = 1):
    if reps in _BUILD_CACHE:
        return _BUILD_CACHE[reps]

    import concourse.bass as bass  # noqa: F401
    import concourse.mybir as mybir
    from concourse import bacc
    from concourse.tile import TileContext
    from concourse.masks import make_identity
    from contextlib import ExitStack

    f32 = mybir.dt.float32
    bf16 = mybir.dt.bfloat16
    AF = mybir.ActivationFunctionType
    OP = mybir.AluOpType

    nc = bacc.Bacc(num_devices=8)

    xT = nc.declare_dram_parameter("xT", [D_MODEL, SEQ], bf16, isOutput=False)
    wdq = nc.declare_dram_parameter("wdq", [D_MODEL, D_KV], bf16, isOutput=False)
    wdkv = nc.declare_dram_parameter("wdkv", [D_MODEL, D_KV], bf16, isOutput=False)
    wq = nc.declare_dram_parameter("wq", [D_KV, H_LOC * 128], bf16, isOutput=False)
    wuk2 = nc.declare_dram_parameter("wuk2", [D_KV, 2 * 128], bf16, isOutput=False)
    wkr2 = nc.declare_dram_parameter("wkr2", [D_MODEL, 2 * 128], bf16, isOutput=False)
    wuv = nc.declare_dram_parameter("wuv", [D_KV, H_LOC * 128], bf16, isOutput=False)
    wout = nc.declare_dram_parameter("wout", [H_LOC * 128, D_MODEL], bf16, isOutput=False)
    mult = nc.declare_dram_parameter("mult", [128, 2 * SEQ], bf16, isOutput=False)
    masks = nc.declare_dram_parameter("masks", [128, 4 * 512], bf16, isOutput=False)
    y = nc.declare_dram_parameter("y", [SEQ, D_MODEL], bf16, isOutput=True)

    SCALE = 1.0 / float(np.sqrt(np.float32(D_HEAD)))
    NKT = D_MODEL // 128
    NLT = D_KV // 128
    NSN = SEQ // 512
    VROW = D_HEAD + 1

    with TileContext(nc) as tc, ExitStack() as top:
        pp = top.enter_context(tc.tile_pool(name="persist", bufs=1))
        kt_sb = pp.tile([128, H_LOC * SEQ], bf16, tag="kt")
        v_sb = pp.tile([128, (SEQ // 128) * H_LOC * VROW], bf16, tag="v")
        wdq_sb = pp.tile([128, NKT * D_KV], bf16, tag="wdq")
        wdkv_sb = pp.tile([128, NKT * D_KV], bf16, tag="wdkv")
        wkr2_sb = pp.tile([128, NKT * 256], bf16, tag="wkr2")
        wq_sb = pp.tile([128, NLT * 512], bf16, tag="wq")
        wuk2_sb = pp.tile([128, NLT * 256], bf16, tag="wuk2")
        wuv_sb = pp.tile([128, NLT * 512], bf16, tag="wuv")
        wout_sb = pp.tile([128, NLT * D_MODEL], bf16, tag="wout")
        mult_sb = pp.tile([128, 2 * SEQ], bf16, tag="mult")
        masks_sb = pp.tile([128, 4 * 512], bf16, tag="masks")
        ident_sb = pp.tile([128, 128], bf16, tag="ident")
        ones_sb = pp.tile([128, 1], bf16, tag="ones")

        for _rep in range(reps):
            with ExitStack() as body:
                pX = body.enter_context(tc.tile_pool(name="pX", bufs=1))
                pCq = body.enter_context(tc.tile_pool(name="pCq", bufs=2))
                pCkv = body.enter_context(tc.tile_pool(name="pCkv", bufs=2))
                pCp = body.enter_context(tc.tile_pool(name="pCp", bufs=3))
                pSq = body.enter_context(tc.tile_pool(name="pSq", bufs=2))
                pSt = body.enter_context(tc.tile_pool(name="pSt", bufs=3))
                pBs = body.enter_context(tc.tile_pool(name="pBs", bufs=1))
                pQt = body.enter_context(tc.tile_pool(name="pQt", bufs=2))
                pEs = body.enter_context(tc.tile_pool(name="pEs", bufs=8))
                pOd = body.enter_context(tc.tile_pool(name="pOd", bufs=4))
                pOt = body.enter_context(tc.tile_pool(name="pOt", bufs=4))
                pYs = body.enter_context(tc.tile_pool(name="pYs", bufs=2))
                psP = body.enter_context(tc.tile_pool(name="psP", bufs=3, space="PSUM"))
                psX = body.enter_context(tc.tile_pool(name="psX", bufs=2, space="PSUM"))

                # ---- weight / constant loads; wdq+wdkv+x(chunk0) ride the two
                # fast HWDGE queues, everything else on gpsimd ----
                xch0a = pX.tile([128, 8 * 512], bf16, tag="xsa", name="xs0a")
                xch0b = pX.tile([128, 8 * 512], bf16, tag="xsb", name="xs0b")
                nc.gpsimd.dma_start(out=mult_sb[:], in_=mult[:, :])
                for lt in range(NLT):
                    nc.gpsimd.dma_start(out=wq_sb[:, lt * 512:(lt + 1) * 512],
                                        in_=wq[lt * 128:(lt + 1) * 128, :])
                    nc.gpsimd.dma_start(out=wuk2_sb[:, lt * 256:(lt + 1) * 256],
                                        in_=wuk2[lt * 128:(lt + 1) * 128, :])
                    nc.gpsimd.dma_start(out=wuv_sb[:, lt * 512:(lt + 1) * 512],
                                        in_=wuv[lt * 128:(lt + 1) * 128, :])
                nc.gpsimd.dma_start(out=masks_sb[:], in_=masks[:, :])
                for kt in range(NKT):
                    nc.sync.dma_start(out=wdq_sb[:, kt * D_KV:(kt + 1) * D_KV],
                                      in_=wdq[kt * 128:(kt + 1) * 128, :])
                    eng = (nc.sync, nc.scalar)[kt % 2]
                    _xd = (xch0a, xch0b)[kt // 8]
                    eng.dma_start(out=_xd[:, (kt % 8) * 512:(kt % 8 + 1) * 512],
                                  in_=xT[kt * 128:(kt + 1) * 128, 0:512])
                    nc.scalar.dma_start(out=wdkv_sb[:, kt * D_KV:(kt + 1) * D_KV],
                                        in_=wdkv[kt * 128:(kt + 1) * 128, :])
                    nc.gpsimd.dma_start(out=wkr2_sb[:, kt * 256:(kt + 1) * 256],
                                        in_=wkr2[kt * 128:(kt + 1) * 128, :])
                for f in range(NLT):
                    nc.scalar.dma_start(out=wout_sb[:, f * D_MODEL:(f + 1) * D_MODEL],
                                        in_=wout[f * 128:(f + 1) * 128, :])
                nc.gpsimd.memset(ones_sb[:], 1.0)
                make_identity(nc, ident_sb[:])
                v_ones = v_sb.rearrange("p (k d) -> p k d", d=VROW)[:, :, 128:129]
                nc.vector.memset(v_ones, 1.0)

                for sn in range(NSN):
                    s0, s1 = sn * 512, (sn + 1) * 512

                    if sn == 0:
                        xcha, xchb = xch0a, xch0b
                    else:
                        xcha = pX.tile([128, 8 * 512], bf16, tag="xsa", name=f"xs{sn}a")
                        xchb = pX.tile([128, 8 * 512], bf16, tag="xsb", name=f"xs{sn}b")
                        for kt in range(NKT):
                            eng = (nc.sync, nc.scalar)[kt % 2]
                            _xd = (xcha, xchb)[kt // 8]
                            eng.dma_start(out=_xd[:, (kt % 8) * 512:(kt % 8 + 1) * 512],
                                          in_=xT[kt * 128:(kt + 1) * 128, s0:s1])
                    xs = [(xcha, xchb)[kt // 8][:, (kt % 8) * 512:(kt % 8 + 1) * 512]
                          for kt in range(NKT)]

                    # ---- latents c_q / c_kv, rmsnorm (lt chains in pairs) ----
                    cq_cur = pCq.tile([128, NLT * 512], bf16, tag="cq", name=f"cq{sn}")
                    ckv_cur = pCkv.tile([128, NLT * 512], bf16, tag="ckv", name=f"ckv{sn}")
                    for ci, (cname, wd_sb, cfull) in enumerate(
                            (("q", wdq_sb, cq_cur), ("kv", wdkv_sb, ckv_cur))):
                        cps_l, sq_l = [], []
                        for lp in range(2):  # lt pairs (0,1) and (2,3)
                            cp = psP.tile([128, 1024], f32, tag="mm2",
                                          name=f"c{cname}{sn}_{lp}")
                            for u in range(2):
                                lt = 2 * lp + u
                                for kt in range(NKT):
                                    nc.tensor.matmul(
                                        cp[:, u * 512:(u + 1) * 512],
                                        wd_sb[:, kt * D_KV + lt * 128: kt * D_KV + (lt + 1) * 128],
                                        xs[kt],
                                        start=(kt == 0), stop=(kt == NKT - 1))
                            cps = pCp.tile([128, 1024], f32, tag="cpre",
                                           name=f"cp{cname}{sn}_{lp}")
                            nc.vector.tensor_copy(cps[:], cp[:])
                            sq = pSq.tile([128, 1024], bf16, tag="sq",
                                          name=f"sq{cname}{sn}_{lp}")
                            nc.vector.tensor_tensor(sq[:], cps[:], cps[:], OP.mult)
                            cps_l.append(cps)
                            sq_l.append(sq)
                        ss = psX.tile([128, 512], f32, tag="x", name=f"ss{cname}{sn}")
                        for lp in range(2):
                            for u in range(2):
                                nc.tensor.matmul(
                                    ss[0:1, :], ones_sb[:],
                                    sq_l[lp][:, u * 512:(u + 1) * 512],
                                    start=(lp == 0 and u == 0),
                                    stop=(lp == 1 and u == 1))
                        # rstd = (ss/512 + eps)^-0.5 via Newton on DVE (v ~ 1):
                        # y0 = 1 -> y1 = 1.5 - 0.5 v, then two more iterations.
                        v_t = pSt.tile([1, 512], f32, tag="st1")
                        nc.vector.tensor_scalar(v_t[:], ss[0:1, :], 1.0 / D_KV, EPS,
                                                OP.mult, OP.add)
                        yy = pSt.tile([1, 512], f32, tag="st1")
                        nc.vector.tensor_scalar(yy[:], ss[0:1, :], -0.5 / D_KV,
                                                1.5 - 0.5 * EPS, OP.mult, OP.add)
                        tsq = pSt.tile([1, 512], f32, tag="st1")
                        for _it in range(2):
                            nc.vector.tensor_tensor(tsq[:], yy[:], yy[:], OP.mult)
                            nc.vector.tensor_tensor(tsq[:], tsq[:], v_t[:], OP.mult)
                            nc.vector.tensor_scalar(tsq[:], tsq[:], -0.5, 1.5,
                                                    OP.mult, OP.add)
                            nc.vector.tensor_tensor(yy[:], yy[:], tsq[:], OP.mult)
                        bstd = pBs.tile([128, 1024], f32, tag="bstd")
                        nc.gpsimd.partition_broadcast(bstd[:, 0:512], yy[:])
                        nc.gpsimd.partition_broadcast(bstd[:, 512:1024], yy[:])
                        for lp in range(2):
                            nc.vector.tensor_tensor(
                                cfull[:, lp * 1024:(lp + 1) * 1024],
                                cps_l[lp][:], bstd[:], OP.mult)

                    # ---- k_rope (p=0,1 chains in one pair tile) ----
                    kp = psP.tile([128, 1024], f32, tag="mm2", name=f"kr{sn}")
                    for p in range(2):
                        for kt in range(NKT):
                            nc.tensor.matmul(
                                kp[:, p * 512:(p + 1) * 512],
                                wkr2_sb[:, kt * 256 + p * 128: kt * 256 + (p + 1) * 128],
                                xs[kt],
                                start=(kt == 0), stop=(kt == NKT - 1))
                    m0 = mult_sb[:, 0 * SEQ + s0: 0 * SEQ + s1]
                    m1 = mult_sb[:, 1 * SEQ + s0: 1 * SEQ + s1]
                    for p in range(2):
                        h0, h1 = 2 * p, 2 * p + 1
                        k0 = kt_sb[:, h0 * SEQ + s0: h0 * SEQ + s1]
                        k1 = kt_sb[:, h1 * SEQ + s0: h1 * SEQ + s1]
                        nc.vector.tensor_tensor(k0[64:128, :], kp[64:128, p * 512:(p + 1) * 512],
                                                m0[64:128, :], OP.mult)
                        nc.vector.tensor_tensor(k1[0:64, :], kp[0:64, p * 512:(p + 1) * 512],
                                                m1[0:64, :], OP.mult)

                    # ---- q/k/v up-projections ----
                    def cnq(lt):
                        return cq_cur[:, lt * 512:(lt + 1) * 512]

                    def cnkv(lt):
                        return ckv_cur[:, lt * 512:(lt + 1) * 512]

                    qt_cur = pQt.tile([128, H_LOC * 512], bf16, tag="qt", name=f"qt{sn}")
                    for hp in range(2):  # hl pairs (0,1) and (2,3)
                        qp = psP.tile([128, 1024], f32, tag="mm2", name=f"qp{sn}_{hp}")
                        for u in range(2):
                            hl = 2 * hp + u
                            for lt in range(NLT):
                                nc.tensor.matmul(
                                    qp[:, u * 512:(u + 1) * 512],
                                    wq_sb[:, lt * 512 + hl * 128: lt * 512 + (hl + 1) * 128],
                                    cnq(lt),
                                    start=(lt == 0), stop=(lt == NLT - 1))
                        # multiplier: [mult block (hl%2)] for u=0,1 -> 3D AP
                        qdst = qt_cur.rearrange("p (a c) -> p a c", c=512)[
                            :, 2 * hp: 2 * hp + 2, :]
                        qsrc = qp.rearrange("p (a c) -> p a c", c=512)
                        mm3 = mult_sb.rearrange("p (a c) -> p a c", c=SEQ)[
                            :, :, s0:s1]
                        nc.vector.tensor_tensor(qdst, qsrc, mm3, OP.mult)

                    up = psP.tile([128, 1024], f32, tag="mm2", name=f"uk{sn}")
                    for p in range(2):
                        for lt in range(NLT):
                            nc.tensor.matmul(
                                up[:, p * 512:(p + 1) * 512],
                                wuk2_sb[:, lt * 256 + p * 128: lt * 256 + (p + 1) * 128],
                                cnkv(lt),
                                start=(lt == 0), stop=(lt == NLT - 1))
                    for p in range(2):
                        h0, h1 = 2 * p, 2 * p + 1
                        k0 = kt_sb[:, h0 * SEQ + s0: h0 * SEQ + s1]
                        k1 = kt_sb[:, h1 * SEQ + s0: h1 * SEQ + s1]
                        nc.vector.tensor_tensor(k0[0:64, :], up[0:64, p * 512:(p + 1) * 512],
                                                m0[0:64, :], OP.mult)
                        nc.vector.tensor_tensor(k1[64:128, :], up[64:128, p * 512:(p + 1) * 512],
                                                m1[64:128, :], OP.mult)

                    for sp_i in range(2):  # st pairs
                        vp = psP.tile([128, 1024], f32, tag="mm2", name=f"vp{sn}_{sp_i}")
                        for u in range(2):
                            st = 2 * sp_i + u
                            for lt in range(NLT):
                                nc.tensor.matmul(
                                    vp[:, u * 512:(u + 1) * 512],
                                    cnkv(lt)[:, st * 128:(st + 1) * 128],
                                    wuv_sb[:, lt * 512:(lt + 1) * 512],
                                    start=(lt == 0), stop=(lt == NLT - 1))
                        s_tile = sn * 4 + 2 * sp_i
                        vdst = v_sb.rearrange("p (k d) -> p k d", d=VROW)[
                            :, s_tile * H_LOC:(s_tile + 2) * H_LOC, 0:128]
                        vsrc = vp.rearrange("p (k d) -> p k d", d=128)
                        nc.vector.tensor_copy(vdst, vsrc)

                    # ---- attention for q-block qb == sn ----
                    qb = sn
                    njt = (qb + 1) * 4
                    otc = [pOt.tile([128, 512], bf16, tag="otc", name=f"otc{qb}_{f}")
                           for f in range(H_LOC)]
                    for hl in range(H_LOC):
                        # scores + exp, j-tiles in pairs; es tiles persist
                        es_l = {}
                        npair = njt // 2
                        for pr in range(npair):
                            j0 = 2 * pr
                            kd0 = j0 - qb * 4
                            sp = psP.tile([128, 1024], f32, tag="mm2",
                                          name=f"s{qb}{hl}{pr}")
                            for u in range(2):
                                jt = j0 + u
                                kd = jt - qb * 4
                                c0 = max(kd, 0) * 128
                                nc.tensor.matmul(
                                    sp[:, u * 512 + c0:(u + 1) * 512],
                                    kt_sb[:, hl * SEQ + jt * 128: hl * SEQ + (jt + 1) * 128],
                                    qt_cur[:, hl * 512 + c0:(hl + 1) * 512],
                                    start=True, stop=True)
                            es = pEs.tile([128, 1024], bf16, tag="expS",
                                          name=f"e{qb}{hl}{pr}")
                            if kd0 < 0:
                                # full pair: one exp over both halves
                                nc.scalar.activation(es[:], sp[:], AF.Exp, scale=SCALE)
                            else:
                                # diag pair: exp only the live slices
                                for u in range(2):
                                    c0 = (kd0 + u) * 128
                                    nc.scalar.activation(
                                        es[:, u * 512 + c0:(u + 1) * 512],
                                        sp[:, u * 512 + c0:(u + 1) * 512],
                                        AF.Exp, scale=SCALE)
                            for u in range(2):
                                kd = j0 + u - qb * 4
                                if kd >= 0:
                                    c0 = kd * 128
                                    nc.vector.tensor_tensor(
                                        es[:, u * 512 + c0:(u + 1) * 512],
                                        es[:, u * 512 + c0:(u + 1) * 512],
                                        masks_sb[:, kd * 512 + c0:(kd + 1) * 512],
                                        OP.mult)
                            es_l[pr] = es
                        # AV: qs-major single-bank chains with ones-column V
                        for qs in range(4):
                            ab = psX.tile([128, 512], f32, tag="x",
                                          name=f"ab{qb}{hl}{qs}")
                            for jt in range(qb * 4 + qs + 1):
                                kd = jt - qb * 4
                                es = es_l[jt // 2]
                                u = jt % 2
                                vsl = v_sb[:, jt * H_LOC * VROW + hl * VROW:
                                           jt * H_LOC * VROW + (hl + 1) * VROW]
                                nc.tensor.matmul(
                                    ab[:, 0:VROW],
                                    es[:, u * 512 + qs * 128: u * 512 + (qs + 1) * 128],
                                    vsl,
                                    start=(jt == 0), stop=(jt == qb * 4 + qs))
                            zr = pOd.tile([128, 1], f32, tag="zr")
                            nc.vector.reciprocal(zr[:], ab[:, 128:VROW])
                            od = pOd.tile([128, 128], bf16, tag="od")
                            nc.vector.tensor_scalar_mul(od[:], ab[:, 0:128], zr[:])
                            tp = psP.tile([128, 1024], bf16, tag="mm2",
                                          name=f"tp{qb}{hl}{qs}")
                            nc.tensor.transpose(tp[:, 0:128], od[:], ident_sb[:])
                            nc.vector.tensor_copy(otc[hl][:, qs * 128:(qs + 1) * 128],
                                                  tp[:, 0:128])

                    # ---- output projection for this q-block (ncol pairs) ----
                    for st in range(4):
                        row0 = qb * 512 + st * 128
                        for np_i in range(2):
                            yp = psP.tile([128, 1024], f32, tag="mm2",
                                          name=f"y{qb}{st}{np_i}")
                            for u in range(2):
                                ncol = 2 * np_i + u
                                for f in range(H_LOC):
                                    nc.tensor.matmul(
                                        yp[:, u * 512:(u + 1) * 512],
                                        otc[f][:, st * 128:(st + 1) * 128],
                                        wout_sb[:, f * D_MODEL + ncol * 512:
                                                f * D_MODEL + (ncol + 1) * 512],
                                        start=(f == 0), stop=(f == H_LOC - 1))
                            ys = pYs.tile([128, 1024], bf16, tag="ysb")
                            if np_i == 0:
                                nc.vector.tensor_copy(ys[:], yp[:])
                            else:
                                nc.scalar.copy(ys[:], yp[:])
                            nc.sync.dma_start(
                                out=y[row0:row0 + 128, np_i * 1024:(np_i + 1) * 1024],
                                in_=ys[:])

    nc.finalize()
    _BUILD_CACHE[reps] = nc
    return nc


def _rope_mult():
    half = D_ROPE // 2
    theta = 1.0 / (ROPE_BASE ** (np.arange(0, D_HEAD, 2, dtype=np.float32) / D_HEAD))
    idx = np.arange(SEQ, dtype=np.float32)[:, None] * theta[None, :]
    r = np.tile(np.cos(idx[:, :half]), (1, 2)) + np.tile(np.sin(idx[:, :half]), (1, 2))
    return np.ascontiguousarray(r.T).astype(np.float32)  # [64, SEQ]


def make_inputs(x, W_dq, W_uq, W_dkv, W_uk, W_uv, W_qr, W_kr, g_q, g_kv, W_out, b_out):
    rT = _rope_mult()
    mult = np.empty((128, 2 * SEQ), np.float32)
    mult[0:64, 0:SEQ] = 1.0
    mult[64:128, 0:SEQ] = rT
    mult[0:64, SEQ:] = rT
    mult[64:128, SEQ:] = 1.0
    mult = mult.astype(BF16)

    masks = np.zeros((128, 4 * 512), np.float32)
    jl = np.arange(128)[:, None]
    ql = np.arange(512)[None, :]
    for k in range(4):
        masks[:, k * 512:(k + 1) * 512] = (ql >= 128 * k + jl)
    masks = masks.astype(BF16)

    gq = g_q.astype(np.float32)[:, None]
    gkv = g_kv.astype(np.float32)[:, None]
    Wuq_g = W_uq * gq
    Wqr_g = W_qr * gq
    Wuk_g = W_uk * gkv
    Wuv_g = W_uv * gkv

    in_maps = []
    for core in range(N_CORES):
        b = core // 4
        g = core % 4
        heads = [4 * g + i for i in range(H_LOC)]

        xb = np.ascontiguousarray(x[b].T).astype(BF16)  # [d_model, seq]

        wq_pack = np.empty((D_KV, H_LOC * 128), np.float32)
        for hl, h in enumerate(heads):
            a = Wuq_g[:, h * 64:(h + 1) * 64]
            r = Wqr_g[:, h * 64:(h + 1) * 64]
            blk = np.concatenate([a, r], axis=1) if hl % 2 == 0 else np.concatenate([r, a], axis=1)
            wq_pack[:, hl * 128:(hl + 1) * 128] = blk

        wuk2 = np.empty((D_KV, 256), np.float32)
        wkr2 = np.empty((D_MODEL, 256), np.float32)
        for p in range(2):
            h0, h1 = heads[2 * p], heads[2 * p + 1]
            wuk2[:, p * 128: p * 128 + 64] = Wuk_g[:, h0 * 64:(h0 + 1) * 64]
            wuk2[:, p * 128 + 64: p * 128 + 128] = Wuk_g[:, h1 * 64:(h1 + 1) * 64]
            wkr2[:, p * 128: p * 128 + 64] = W_kr[:, h1 * 64:(h1 + 1) * 64]
            wkr2[:, p * 128 + 64: p * 128 + 128] = W_kr[:, h0 * 64:(h0 + 1) * 64]

        wuv_pack = np.concatenate(
            [Wuv_g[:, h * 128:(h + 1) * 128] for h in heads], axis=1)
        wout_pack = np.concatenate(
            [W_out[h * 128:(h + 1) * 128, :] for h in heads], axis=0)

        in_maps.append({
            "xT": xb,
            "wdq": W_dq.astype(BF16),
            "wdkv": W_dkv.astype(BF16),
            "wq": wq_pack.astype(BF16),
            "wuk2": wuk2.astype(BF16),
            "wkr2": wkr2.astype(BF16),
            "wuv": wuv_pack.astype(BF16),
            "wout": wout_pack.astype(BF16),
            "mult": mult,
            "masks": masks,
        })
    return in_maps


def kernel(**inputs):
    inputs = {k: np.asarray(v) for k, v in inputs.items()}
    in_maps = make_inputs(
        inputs["x"], inputs["W_dq"], inputs["W_uq"], inputs["W_dkv"],
        inputs["W_uk"], inputs["W_uv"], inputs["W_qr"], inputs["W_kr"],
        inputs["g_q"], inputs["g_kv"], inputs["W_out"], inputs["b_out"])

    nc = build_program(reps=1)
    from concourse.bass_utils import run_bass_kernel_spmd
    res = run_bass_kernel_spmd(nc, in_maps, list(range(N_CORES)))

    b_out = inputs["b_out"].astype(np.float32)
    out = np.zeros((BATCH, SEQ, D_MODEL), np.float32)
    for core in range(N_CORES):
        out[core // 4] += res.results[core]["y"].astype(np.float32)
    out += b_out[None, None, :]
    return out


# revision 5
# speedup vs baseline: 2.2091x; 1.2264x over previous
"""MLA (multi-head latent attention) Trainium2 Bass kernel, v3.

Sharding: 8 cores = batch(2) x head-groups(4 heads each). Latent
projections replicated per core (no collective -> no mesh sync, immune to
pipelined-execution desync). One interleaved pipeline over 4 sequence
chunks: latents(sn)+k_rope(sn) -> q/k/v up-proj(sn) -> attention(qb=sn).

v3 packs almost all PSUM work into [128,1024] two-bank pair tiles (one
accumulation group per bank), halving instruction counts on PE/Act/DVE:
exp runs once per score PAIR, latent drains copy 1024 columns at a time,
and attention AV runs qs-major in single-bank chains so the softmax
denominator still folds into the PE via a ones-column on V.

rmsnorm's rsqrt is computed with a 3-step Newton iteration on the DVE
(v ~ 1), keeping the Act engine exp-only: one activation table load total.
"""

import sys
import numpy as np
import ml_dtypes

for _p in ("/opt/trn_rl_repo", "/root/.axon_site/_ro/trn_rl_repo"):
    if _p not in sys.path:
        sys.path.append(_p)

BF16 = ml_dtypes.bfloat16

D_MODEL = 2048
SEQ = 2048
BATCH = 2
N_HEADS = 16
D_HEAD = 128
D_KV = 512
D_ROPE = 64
ROPE_BASE = 10000.0
EPS = 1e-5
H_LOC = 4          # heads per core
N_CORES = 8

_BUILD_CACHE = {}


def build_program(reps: int = 1):
    if reps in _BUILD_CACHE:
        return _BUILD_CACHE[reps]

    import concourse.bass as bass  # noqa: F401
    import concourse.mybir as mybir
    from concourse import bacc
    from concourse.tile import TileContext
    from concourse.masks import make_identity
    from contextlib import ExitStack

    f32 = mybir.dt.float32
    bf16 = mybir.dt.bfloat16
    AF = mybir.ActivationFunctionType
    OP = mybir.AluOpType

    nc = bacc.Bacc(num_devices=8)

    xT = nc.declare_dram_parameter("xT", [D_MODEL, SEQ], bf16, isOutput=False)
    wdq = nc.declare_dram_parameter("wdq", [D_MODEL, D_KV], bf16, isOutput=False)
    wdkv = nc.declare_dram_parameter("wdkv", [D_MODEL, D_KV], bf16, isOutput=False)
    wq = nc.declare_dram_parameter("wq", [D_KV, H_LOC * 128], bf16, isOutput=False)
    wuk2 = nc.declare_dram_parameter("wuk2", [D_KV, 2 * 128], bf16, isOutput=False)
    wkr2 = nc.declare_dram_parameter("wkr2", [D_MODEL, 2 * 128], bf16, isOutput=False)
    wuv = nc.declare_dram_parameter("wuv", [D_KV, H_LOC * 128], bf16, isOutput=False)
    wout = nc.declare_dram_parameter("wout", [H_LOC * 128, D_MODEL], bf16, isOutput=False)
    mult = nc.declare_dram_parameter("mult", [128, 2 * SEQ], bf16, isOutput=False)
    masks = nc.declare_dram_parameter("masks", [128, 4 * 512], bf16, isOutput=False)
    y = nc.declare_dram_parameter("y", [SEQ, D_MODEL], bf16, isOutput=True)

    SCALE = 1.0 / float(np.sqrt(np.float32(D_HEAD)))
    NKT = D_MODEL // 128
    NLT = D_KV // 128
    NSN = SEQ // 512
    VROW = D_HEAD + 1

    with TileContext(nc) as tc, ExitStack() as top:
        pp = top.enter_context(tc.tile_pool(name="persist", bufs=1))
        kt_sb = pp.tile([128, H_LOC * SEQ], bf16, tag="kt")
        v_sb = pp.tile([128, (SEQ // 128) * H_LOC * VROW], bf16, tag="v")
        wdq_sb = pp.tile([128, NKT * D_KV], bf16, tag="wdq")
        wdkv_sb = pp.tile([128, NKT * D_KV], bf16, tag="wdkv")
        wkr2_sb = pp.tile([128, NKT * 256], bf16, tag="wkr2")
        wq_sb = pp.tile([128, NLT * 512], bf16, tag="wq")
        wuk2_sb = pp.tile([128, NLT * 256], bf16, tag="wuk2")
        wuv_sb = pp.tile([128, NLT * 512], bf16, tag="wuv")
        wout_sb = pp.tile([128, NLT * D_MODEL], bf16, tag="wout")
        mult_sb = pp.tile([128, 2 * SEQ], bf16, tag="mult")
        masks_sb = pp.tile([128, 4 * 512], bf16, tag="masks")
        ident_sb = pp.tile([128, 128], bf16, tag="ident")
        ones_sb = pp.tile([128, 1], bf16, tag="ones")

        for _rep in range(reps):
            with ExitStack() as body:
                pX = body.enter_context(tc.tile_pool(name="pX", bufs=1))
                pCq = body.enter_context(tc.tile_pool(name="pCq", bufs=2))
                pCkv = body.enter_context(tc.tile_pool(name="pCkv", bufs=2))
                pCp = body.enter_context(tc.tile_pool(name="pCp", bufs=3))
                pSq = body.enter_context(tc.tile_pool(name="pSq", bufs=2))
                pSt = body.enter_context(tc.tile_pool(name="pSt", bufs=3))
                pBs = body.enter_context(tc.tile_pool(name="pBs", bufs=1))
                pQt = body.enter_context(tc.tile_pool(name="pQt", bufs=2))
                pEs = body.enter_context(tc.tile_pool(name="pEs", bufs=8))
                pOd = body.enter_context(tc.tile_pool(name="pOd", bufs=4))
                pOt = body.enter_context(tc.tile_pool(name="pOt", bufs=4))
                pYs = body.enter_context(tc.tile_pool(name="pYs", bufs=2))
                psP = body.enter_context(tc.tile_pool(name="psP", bufs=3, space="PSUM"))
                psX = body.enter_context(tc.tile_pool(name="psX", bufs=2, space="PSUM"))

                # ---- weight / constant loads; wdq+wdkv+x(chunk0) ride the two
                # fast HWDGE queues, everything else on gpsimd ----
                xch0a = pX.tile([128, 8 * 512], bf16, tag="xsa", name="xs0a")
                xch0b = pX.tile([128, 8 * 512], bf16, tag="xsb", name="xs0b")
                xT3 = xT.rearrange("(k p) c -> p k c", p=128)  # [128, 16, 2048]
                nc.gpsimd.dma_start(out=mult_sb[:], in_=mult[:, :])
                for lt in range(NLT):
                    nc.gpsimd.dma_start(out=wq_sb[:, lt * 512:(lt + 1) * 512],
                                        in_=wq[lt * 128:(lt + 1) * 128, :])
                    nc.gpsimd.dma_start(out=wuk2_sb[:, lt * 256:(lt + 1) * 256],
                                        in_=wuk2[lt * 128:(lt + 1) * 128, :])
                    nc.gpsimd.dma_start(out=wuv_sb[:, lt * 512:(lt + 1) * 512],
                                        in_=wuv[lt * 128:(lt + 1) * 128, :])
                nc.gpsimd.dma_start(out=masks_sb[:], in_=masks[:, :])
                for kt in range(NKT):
                    nc.sync.dma_start(out=wdq_sb[:, kt * D_KV:(kt + 1) * D_KV],
                                      in_=wdq[kt * 128:(kt + 1) * 128, :])
                    eng = (nc.sync, nc.scalar)[kt % 2]
                    _xd = (xch0a, xch0b)[kt // 8]
                    eng.dma_start(out=_xd[:, (kt % 8) * 512:(kt % 8 + 1) * 512],
                                  in_=xT[kt * 128:(kt + 1) * 128, 0:512])
                    nc.scalar.dma_start(out=wdkv_sb[:, kt * D_KV:(kt + 1) * D_KV],
                                        in_=wdkv[kt * 128:(kt + 1) * 128, :])
                    nc.gpsimd.dma_start(out=wkr2_sb[:, kt * 256:(kt + 1) * 256],
                                        in_=wkr2[kt * 128:(kt + 1) * 128, :])
                nc.scalar.dma_start(
                    out=wout_sb.rearrange("p (k c) -> p k c", c=D_MODEL),
                    in_=wout.rearrange("(k p) c -> p k c", p=128))
                nc.gpsimd.memset(ones_sb[:], 1.0)
                make_identity(nc, ident_sb[:])
                v_ones = v_sb.rearrange("p (k d) -> p k d", d=VROW)[:, :, 128:129]
                nc.vector.memset(v_ones, 1.0)

                xch_tiles = {0: (xch0a, xch0b)}

                def prefetch_x(snn):
                    xcha = pX.tile([128, 8 * 512], bf16, tag="xsa", name=f"xs{snn}a")
                    xchb = pX.tile([128, 8 * 512], bf16, tag="xsb", name=f"xs{snn}b")
                    t0, t1 = snn * 512, (snn + 1) * 512
                    nc.sync.dma_start(
                        out=xcha.rearrange("p (k c) -> p k c", c=512),
                        in_=xT3[:, 0:8, t0:t1])
                    nc.scalar.dma_start(
                        out=xchb.rearrange("p (k c) -> p k c", c=512),
                        in_=xT3[:, 8:16, t0:t1])
                    xch_tiles[snn] = (xcha, xchb)

                for sn in range(NSN):
                    s0, s1 = sn * 512, (sn + 1) * 512

                    xcha, xchb = xch_tiles[sn]
                    xs = [(xcha, xchb)[kt // 8][:, (kt % 8) * 512:(kt % 8 + 1) * 512]
                          for kt in range(NKT)]

                    # ---- latents c_q / c_kv, rmsnorm (lt chains in pairs) ----
                    cq_cur = pCq.tile([128, NLT * 512], bf16, tag="cq", name=f"cq{sn}")
                    ckv_cur = pCkv.tile([128, NLT * 512], bf16, tag="ckv", name=f"ckv{sn}")
                    for ci, (cname, wd_sb, cfull) in enumerate(
                            (("q", wdq_sb, cq_cur), ("kv", wdkv_sb, ckv_cur))):
                        cps_l, sq_l = [], []
                        for lp in range(2):  # lt pairs (0,1) and (2,3)
                            cp = psP.tile([128, 1024], f32, tag="mm2",
                                          name=f"c{cname}{sn}_{lp}")
                            for u in range(2):
                                lt = 2 * lp + u
                                for kt in range(NKT):
                                    nc.tensor.matmul(
                                        cp[:, u * 512:(u + 1) * 512],
                                        wd_sb[:, kt * D_KV + lt * 128: kt * D_KV + (lt + 1) * 128],
                                        xs[kt],
                                        start=(kt == 0), stop=(kt == NKT - 1))
                            cps = pCp.tile([128, 1024], f32, tag="cpre",
                                           name=f"cp{cname}{sn}_{lp}")
                            nc.vector.tensor_copy(cps[:], cp[:])
                            sq = pSq.tile([128, 1024], bf16, tag="sq",
                                          name=f"sq{cname}{sn}_{lp}")
                            nc.vector.tensor_tensor(sq[:], cps[:], cps[:], OP.mult)
                            cps_l.append(cps)
                            sq_l.append(sq)
                        ss = psX.tile([128, 512], f32, tag="x", name=f"ss{cname}{sn}")
                        for lp in range(2):
                            for u in range(2):
                                nc.tensor.matmul(
                                    ss[0:1, :], ones_sb[:],
                                    sq_l[lp][:, u * 512:(u + 1) * 512],
                                    start=(lp == 0 and u == 0),
                                    stop=(lp == 1 and u == 1))
                        # rstd = (ss/512 + eps)^-0.5 via Newton on DVE (v ~ 1):
                        # y0 = 1 -> y1 = 1.5 - 0.5 v, then two more iterations.
                        v_t = pSt.tile([1, 512], f32, tag="st1")
                        nc.vector.tensor_scalar(v_t[:], ss[0:1, :], 1.0 / D_KV, EPS,
                                                OP.mult, OP.add)
                        yy = pSt.tile([1, 512], f32, tag="st1")
                        nc.vector.tensor_scalar(yy[:], ss[0:1, :], -0.5 / D_KV,
                                                1.5 - 0.5 * EPS, OP.mult, OP.add)
                        tsq = pSt.tile([1, 512], f32, tag="st1")
                        for _it in range(2):
                            nc.vector.tensor_tensor(tsq[:], yy[:], yy[:], OP.mult)
                            nc.vector.tensor_tensor(tsq[:], tsq[:], v_t[:], OP.mult)
                            nc.vector.tensor_scalar(tsq[:], tsq[:], -0.5, 1.5,
                                                    OP.mult, OP.add)
                            nc.vector.tensor_tensor(yy[:], yy[:], tsq[:], OP.mult)
                        bstd = pBs.tile([128, 1024], f32, tag="bstd")
                        nc.gpsimd.partition_broadcast(bstd[:, 0:512], yy[:])
                        nc.gpsimd.partition_broadcast(bstd[:, 512:1024], yy[:])
                        for lp in range(2):
                            nc.vector.tensor_tensor(
                                cfull[:, lp * 1024:(lp + 1) * 1024],
                                cps_l[lp][:], bstd[:], OP.mult)

                    # ---- k_rope (p=0,1 chains in one pair tile) ----
                    kp = psP.tile([128, 1024], f32, tag="mm2", name=f"kr{sn}")
                    for p in range(2):
                        for kt in range(NKT):
                            nc.tensor.matmul(
                                kp[:, p * 512:(p + 1) * 512],
                                wkr2_sb[:, kt * 256 + p * 128: kt * 256 + (p + 1) * 128],
                                xs[kt],
                                start=(kt == 0), stop=(kt == NKT - 1))
                    m0 = mult_sb[:, 0 * SEQ + s0: 0 * SEQ + s1]
                    m1 = mult_sb[:, 1 * SEQ + s0: 1 * SEQ + s1]
                    for p in range(2):
                        h0, h1 = 2 * p, 2 * p + 1
                        k0 = kt_sb[:, h0 * SEQ + s0: h0 * SEQ + s1]
                        k1 = kt_sb[:, h1 * SEQ + s0: h1 * SEQ + s1]
                        nc.vector.tensor_tensor(k0[64:128, :], kp[64:128, p * 512:(p + 1) * 512],
                                                m0[64:128, :], OP.mult)
                        nc.vector.tensor_tensor(k1[0:64, :], kp[0:64, p * 512:(p + 1) * 512],
                                                m1[0:64, :], OP.mult)

                    # prefetch next chunk's x now: its DMAs must ride the
                    # sync/scalar queues AHEAD of this chunk's y writes
                    if sn + 1 < NSN:
                        prefetch_x(sn + 1)

                    # ---- q/k/v up-projections ----
                    def cnq(lt):
                        return cq_cur[:, lt * 512:(lt + 1) * 512]

                    def cnkv(lt):
                        return ckv_cur[:, lt * 512:(lt + 1) * 512]

                    qt_cur = pQt.tile([128, H_LOC * 512], bf16, tag="qt", name=f"qt{sn}")
                    for hp in range(2):  # hl pairs (0,1) and (2,3)
                        qp = psP.tile([128, 1024], f32, tag="mm2", name=f"qp{sn}_{hp}")
                        for u in range(2):
                            hl = 2 * hp + u
                            for lt in range(NLT):
                                nc.tensor.matmul(
                                    qp[:, u * 512:(u + 1) * 512],
                                    wq_sb[:, lt * 512 + hl * 128: lt * 512 + (hl + 1) * 128],
                                    cnq(lt),
                                    start=(lt == 0), stop=(lt == NLT - 1))
                        # multiplier: [mult block (hl%2)] for u=0,1 -> 3D AP
                        qdst = qt_cur.rearrange("p (a c) -> p a c", c=512)[
                            :, 2 * hp: 2 * hp + 2, :]
                        qsrc = qp.rearrange("p (a c) -> p a c", c=512)
                        mm3 = mult_sb.rearrange("p (a c) -> p a c", c=SEQ)[
                            :, :, s0:s1]
                        nc.vector.tensor_tensor(qdst, qsrc, mm3, OP.mult)

                    up = psP.tile([128, 1024], f32, tag="mm2", name=f"uk{sn}")
                    for p in range(2):
                        for lt in range(NLT):
                            nc.tensor.matmul(
                                up[:, p * 512:(p + 1) * 512],
                                wuk2_sb[:, lt * 256 + p * 128: lt * 256 + (p + 1) * 128],
                                cnkv(lt),
                                start=(lt == 0), stop=(lt == NLT - 1))
                    for p in range(2):
                        h0, h1 = 2 * p, 2 * p + 1
                        k0 = kt_sb[:, h0 * SEQ + s0: h0 * SEQ + s1]
                        k1 = kt_sb[:, h1 * SEQ + s0: h1 * SEQ + s1]
                        nc.vector.tensor_tensor(k0[0:64, :], up[0:64, p * 512:(p + 1) * 512],
                                                m0[0:64, :], OP.mult)
                        nc.vector.tensor_tensor(k1[64:128, :], up[64:128, p * 512:(p + 1) * 512],
                                                m1[64:128, :], OP.mult)

                    for sp_i in range(2):  # st pairs
                        vp = psP.tile([128, 1024], f32, tag="mm2", name=f"vp{sn}_{sp_i}")
                        for u in range(2):
                            st = 2 * sp_i + u
                            for lt in range(NLT):
                                nc.tensor.matmul(
                                    vp[:, u * 512:(u + 1) * 512],
                                    cnkv(lt)[:, st * 128:(st + 1) * 128],
                                    wuv_sb[:, lt * 512:(lt + 1) * 512],
                                    start=(lt == 0), stop=(lt == NLT - 1))
                        s_tile = sn * 4 + 2 * sp_i
                        vdst = v_sb.rearrange("p (k d) -> p k d", d=VROW)[
                            :, s_tile * H_LOC:(s_tile + 2) * H_LOC, 0:128]
                        vsrc = vp.rearrange("p (k d) -> p k d", d=128)
                        nc.vector.tensor_copy(vdst, vsrc)

                    # ---- attention for q-block qb == sn ----
                    qb = sn
                    njt = (qb + 1) * 4
                    otc = [pOt.tile([128, 512], bf16, tag="otc", name=f"otc{qb}_{f}")
                           for f in range(H_LOC)]
                    for hl in range(H_LOC):
                        # scores + exp, j-tiles in pairs; es tiles persist
                        es_l = {}
                        npair = njt // 2
                        for pr in range(npair):
                            j0 = 2 * pr
                            kd0 = j0 - qb * 4
                            sp = psP.tile([128, 1024], f32, tag="mm2",
                                          name=f"s{qb}{hl}{pr}")
                            for u in range(2):
                                jt = j0 + u
                                kd = jt - qb * 4
                                c0 = max(kd, 0) * 128
                                nc.tensor.matmul(
                                    sp[:, u * 512 + c0:(u + 1) * 512],
                                    kt_sb[:, hl * SEQ + jt * 128: hl * SEQ + (jt + 1) * 128],
                                    qt_cur[:, hl * 512 + c0:(hl + 1) * 512],
                                    start=True, stop=True)
                            es = pEs.tile([128, 1024], bf16, tag="expS",
                                          name=f"e{qb}{hl}{pr}")
                            if kd0 < 0:
                                # full pair: one exp over both halves
                                nc.scalar.activation(es[:], sp[:], AF.Exp, scale=SCALE)
                            else:
                                # diag pair: exp only the live slices
                                for u in range(2):
                                    c0 = (kd0 + u) * 128
                                    nc.scalar.activation(
                                        es[:, u * 512 + c0:(u + 1) * 512],
                                        sp[:, u * 512 + c0:(u + 1) * 512],
                                        AF.Exp, scale=SCALE)
                            for u in range(2):
                                kd = j0 + u - qb * 4
                                if kd >= 0:
                                    c0 = kd * 128
                                    nc.vector.tensor_tensor(
                                        es[:, u * 512 + c0:(u + 1) * 512],
                                        es[:, u * 512 + c0:(u + 1) * 512],
                                        masks_sb[:, kd * 512 + c0:(kd + 1) * 512],
                                        OP.mult)
                            es_l[pr] = es
                        # AV: qs-major single-bank chains with ones-column V
                        for qs in range(4):
                            ab = psX.tile([128, 512], f32, tag="x",
                                          name=f"ab{qb}{hl}{qs}")
                            for jt in range(qb * 4 + qs + 1):
                                kd = jt - qb * 4
                                es = es_l[jt // 2]
                                u = jt % 2
                                vsl = v_sb[:, jt * H_LOC * VROW + hl * VROW:
                                           jt * H_LOC * VROW + (hl + 1) * VROW]
                                nc.tensor.matmul(
                                    ab[:, 0:VROW],
                                    es[:, u * 512 + qs * 128: u * 512 + (qs + 1) * 128],
                                    vsl,
                                    start=(jt == 0), stop=(jt == qb * 4 + qs))
                            zr = pOd.tile([128, 1], f32, tag="zr")
                            nc.vector.reciprocal(zr[:], ab[:, 128:VROW])
                            od = pOd.tile([128, 128], bf16, tag="od")
                            nc.vector.tensor_scalar_mul(od[:], ab[:, 0:128], zr[:])
                            tp = psP.tile([128, 1024], bf16, tag="mm2",
                                          name=f"tp{qb}{hl}{qs}")
                            nc.tensor.transpose(tp[:, 0:128], od[:], ident_sb[:])
                            nc.vector.tensor_copy(otc[hl][:, qs * 128:(qs + 1) * 128],
                                                  tp[:, 0:128])

                    # ---- output projection for this q-block (ncol pairs) ----
                    for st in range(4):
                        row0 = qb * 512 + st * 128
                        for np_i in range(2):
                            yp = psP.tile([128, 1024], f32, tag="mm2",
                                          name=f"y{qb}{st}{np_i}")
                            for u in range(2):
                                ncol = 2 * np_i + u
                                for f in range(H_LOC):
                                    nc.tensor.matmul(
                                        yp[:, u * 512:(u + 1) * 512],
                                        otc[f][:, st * 128:(st + 1) * 128],
                                        wout_sb[:, f * D_MODEL + ncol * 512:
                                                f * D_MODEL + (ncol + 1) * 512],
                                        start=(f == 0), stop=(f == H_LOC - 1))
                            ys = pYs.tile([128, 1024], bf16, tag="ysb")
                            if np_i == 0:
                                nc.vector.tensor_copy(ys[:], yp[:])
                            else:
                                nc.scalar.copy(ys[:], yp[:])
                            nc.sync.dma_start(
                                out=y[row0:row0 + 128, np_i * 1024:(np_i + 1) * 1024],
                                in_=ys[:])

    nc.finalize()
    _BUILD_CACHE[reps] = nc
    return nc


def _rope_mult():
    half = D_ROPE // 2
    theta = 1.0 / (ROPE_BASE ** (np.arange(0, D_HEAD, 2, dtype=np.float32) / D_HEAD))
    idx = np.arange(SEQ, dtype=np.float32)[:, None] * theta[None, :]
    r = np.tile(np.cos(idx[:, :half]), (1, 2)) + np.tile(np.sin(idx[:, :half]), (1, 2))
    return np.ascontiguousarray(r.T).astype(np.float32)  # [64, SEQ]


def make_inputs(x, W_dq, W_uq, W_dkv, W_uk, W_uv, W_qr, W_kr, g_q, g_kv, W_out, b_out):
    rT = _rope_mult()
    mult = np.empty((128, 2 * SEQ), np.float32)
    mult[0:64, 0:SEQ] = 1.0
    mult[64:128, 0:SEQ] = rT
    mult[0:64, SEQ:] = rT
    mult[64:128, SEQ:] = 1.0
    mult = mult.astype(BF16)

    masks = np.zeros((128, 4 * 512), np.float32)
    jl = np.arange(128)[:, None]
    ql = np.arange(512)[None, :]
    for k in range(4):
        masks[:, k * 512:(k + 1) * 512] = (ql >= 128 * k + jl)
    masks = masks.astype(BF16)

    gq = g_q.astype(np.float32)[:, None]
    gkv = g_kv.astype(np.float32)[:, None]
    Wuq_g = W_uq * gq
    Wqr_g = W_qr * gq
    Wuk_g = W_uk * gkv
    Wuv_g = W_uv * gkv

    in_maps = []
    for core in range(N_CORES):
        b = core // 4
        g = core % 4
        heads = [4 * g + i for i in range(H_LOC)]

        xb = np.ascontiguousarray(x[b].T).astype(BF16)  # [d_model, seq]

        wq_pack = np.empty((D_KV, H_LOC * 128), np.float32)
        for hl, h in enumerate(heads):
            a = Wuq_g[:, h * 64:(h + 1) * 64]
            r = Wqr_g[:, h * 64:(h + 1) * 64]
            blk = np.concatenate([a, r], axis=1) if hl % 2 == 0 else np.concatenate([r, a], axis=1)
            wq_pack[:, hl * 128:(hl + 1) * 128] = blk

        wuk2 = np.empty((D_KV, 256), np.float32)
        wkr2 = np.empty((D_MODEL, 256), np.float32)
        for p in range(2):
            h0, h1 = heads[2 * p], heads[2 * p + 1]
            wuk2[:, p * 128: p * 128 + 64] = Wuk_g[:, h0 * 64:(h0 + 1) * 64]
            wuk2[:, p * 128 + 64: p * 128 + 128] = Wuk_g[:, h1 * 64:(h1 + 1) * 64]
            wkr2[:, p * 128: p * 128 + 64] = W_kr[:, h1 * 64:(h1 + 1) * 64]
            wkr2[:, p * 128 + 64: p * 128 + 128] = W_kr[:, h0 * 64:(h0 + 1) * 64]

        wuv_pack = np.concatenate(
            [Wuv_g[:, h * 128:(h + 1) * 128] for h in heads], axis=1)
        wout_pack = np.concatenate(
            [W_out[h * 128:(h + 1) * 128, :] for h in heads], axis=0)

        in_maps.append({
            "xT": xb,
            "wdq": W_dq.astype(BF16),
            "wdkv": W_dkv.astype(BF16),
            "wq": wq_pack.astype(BF16),
            "wuk2": wuk2.astype(BF16),
            "wkr2": wkr2.astype(BF16),
            "wuv": wuv_pack.astype(BF16),
            "wout": wout_pack.astype(BF16),
            "mult": mult,
            "masks": masks,
        })
    return in_maps


def kernel(**inputs):
    inputs = {k: np.asarray(v) for k, v in inputs.items()}
    in_maps = make_inputs(
        inputs["x"], inputs["W_dq"], inputs["W_uq"], inputs["W_dkv"],
        inputs["W_uk"], inputs["W_uv"], inputs["W_qr"], inputs["W_kr"],
        inputs["g_q"], inputs["g_kv"], inputs["W_out"], inputs["b_out"])

    nc = build_program(reps=1)
    from concourse.bass_utils import run_bass_kernel_spmd
    res = run_bass_kernel_spmd(nc, in_maps, list(range(N_CORES)))

    b_out = inputs["b_out"].astype(np.float32)
    out = np.zeros((BATCH, SEQ, D_MODEL), np.float32)
    for core in range(N_CORES):
        out[core // 4] += res.results[core]["y"].astype(np.float32)
    out += b_out[None, None, :]
    return out
